# revision 43
# baseline (speedup 1.0000x reference)
"""FreeConv2D (locally-connected conv2d + bias) Trainium2 Bass kernel.

out[b,oh,ow,u] = sum_{i,j,c} w[oh,ow,u,i,j,c] * x[b, oh*2+i, ow*2+j, c] + bias[oh,ow,u]

Shapes: x [64,64,64,64], w [30,30,64,5,5,64], b [30,30,64] -> out [64,30,30,64].

Strategy (8 NeuronCores):
  - Shard output rows OH over cores: 4 rows/core (padded 30->32; last 2 dummy).
  - The kernel is DMA-bound (~330 GB/s/core aggregate): w dominates traffic,
    so the w stream is stored as float8_e3m4 * 32 (half the bytes of fp16;
    measured rel err ~1.1e-2 vs the 2e-2 gate) and the output as bf16. The
    matmul mixes lhsT fp16 (x) with rhs fp8e3 (w) — allowed on TRN2.
  - PSUM tiles are not memset: the first matmul into each (oh, role) slot
    uses start=True (even-r blocks are split so the fresh-oh part is its own
    matmul), which keeps the DVE free for drains.
  - Host pre-packs (numpy, not counted in HW time):
      * x    -> per-core fp16 tile [128, 11*32*64]: partition p = dj*64+c for
               column pair (2*mp, 2*mp+1), free = (r, mp, b).
      * w    -> per-core fp8e3 stream [128, TOT] (values * 32): matmul rhs
               blocks in execution order (column-pair taps j in {0,1} / {2,3}
               as K=128 blocks; j=4 taps as K=64 vertically-paired blocks).
      * bias -> per-core fp32 [64, 30*4*64] * 32 replicated over batch
               partitions; host gather divides the 32 back out.
  - Device: 32-phase sweep over column pairs mp. Phase mp:
      * DMA the phase's w blocks (~1 MB).
      * psum phase tile pt[mp] [64, 512] = accum slots (oh_l, role) where
        role 0 = j01-half of loc (oh, mp), role 1 = j23-half of loc (oh, mp-1).
      * matmuls: lhsT = resident x tile [128, 64(b)] (stationary),
        rhs = w blocks [128, N<=512] (moving), accumulate with start=False
        (tiles pre-zeroed by DVE memset; psum has_written semantics make this
        correct whether the first PE write accumulates or overwrites).
      * j=4 taps (K=64) of loc (oh, mp-2) also land in pt[mp-2] role-0 slots.
      * drain loc (.., ow=mp-2): out = pt[ow].role0 + bias + pt[ow+1].role1
        via two DVE tensor_adds into an SBUF staging buffer.
  - One final DMA of staging -> DRAM out [64, 30(ow), 4(oh_l), 64] per core;
    host gathers/transposes/trims to [64, 30, 30, 64].
"""

import os
import sys

import numpy as np

_TRN_REPO = "/opt/trn_rl_repo"
if _TRN_REPO not in sys.path:
    sys.path.insert(0, _TRN_REPO)

# The kernel needs the axon/neuron jax backend; a JAX_PLATFORMS=cpu pin (used
# for reference computation) would hide the NeuronCores. Only effective if jax
# has not been initialized yet in this process.
if "jax" not in sys.modules and "axon" not in os.environ.get("JAX_PLATFORMS", "axon"):
    os.environ.pop("JAX_PLATFORMS", None)

# ---------------- problem constants (hardcoded) ----------------
B, H, W, C = 64, 64, 64, 64
U, K, S = 64, 5, 2
OH = OW = 30
NCORES = 8
NO = 4                      # oh rows per core (padded: 8*4 = 32 >= 30)
OHP = NCORES * NO           # 32
NR = 2 * (NO - 1) + K       # 11 input rows per core
NMP = 32                    # column-pair tiles mp=0..31; also phase count
NT4 = OW // 2               # 15 j4 ow-pairs
HP = 2 * (OHP - 1) + K      # 67 padded input rows overall


def _oh_span(r):
    """Valid local oh range for local input row r: i = r - 2*oh in [0, K-1]."""
    lo = max(0, -(-(r - (K - 1)) // 2))   # ceil((r-4)/2)
    hi = min(NO - 1, r // 2)
    return lo, hi


def build_schedule():
    """Per-phase block lists. Block cols are offsets into the packed w stream.

    Accumulation is single-slot: each output column ow owns one PSUM tile
    PS[ow] [64, NO*U]; every matmul targets the owning tile directly.

    Returns (phases, totcols, wmax) where phases[mp] is a list of dicts:
      kind 'main': K=128 block; role 0 = taps j=(0,1) for ow=mp,
                   role 1 = taps j=(2,3) for ow=mp-1 (separate blocks, each
                   targeting PS[ow]); ncols = noh*64.
      kind 'j4m':  K=128 block for tap j=4, row-pairs: partitions
                   (di, c) = input rows (2*rp+di); serves consumers
                   (oh=rp-1, i=2+di) and (oh=rp, i=di) of ow=mp-1, packed
                   oh-ascending in N; lhsT comes from the x4a tile.
      kind 'j4c':  corner tap (i=4, j=4), K=64, vertically paired in the
                   stream: rows 0:64 = w(oh=q, ow=2*gp), rows 64:128 =
                   w(oh=q, ow=2*gp+1); two matmuls (lhsT from x4b halves),
                   emitted on odd phases mp=2*gp+1; ncols = 64.
    """
    phases = []
    col = 0
    wmax = 0
    for mp in range(NMP):
        blocks = []
        for r in range(NR):
            if mp <= OW:  # main blocks exist for mp=0..30
                lo, hi = _oh_span(r)
                if lo > hi:
                    continue
                noh = hi - lo + 1
                if (mp % 2 == 1 and mp <= OW - 1
                        and os.environ.get("KMERGE", "1") == "1"):
                    # odd phase: ow=mp-1 (j23) and ow=mp (j01) live in the
                    # same bank-aligned PSUM pair tile -> single matmul,
                    # cols ordered (dow, oh, u) with dow0 = ow=mp-1.
                    blocks.append(dict(kind="main2", r=r, mp=mp, col0=col,
                                       ncols=2 * noh * U, oh0=lo, noh=noh))
                    col += 2 * noh * U
                else:
                    for role in (0, 1):
                        ow = mp - role
                        if not (0 <= ow <= OW - 1):
                            continue
                        blocks.append(dict(kind="main", r=r, mp=mp, col0=col,
                                           ncols=noh * U, oh0=lo, noh=noh,
                                           role=role, ow=ow))
                        col += noh * U
        if 1 <= mp <= OW:
            ow = mp - 1
            for rp in range(NO + 1):
                cons = []                    # (oh, i_base), oh ascending
                if 0 <= rp - 1 <= NO - 1:
                    cons.append((rp - 1, 2))
                if rp <= NO - 1:
                    cons.append((rp, 0))
                if cons:
                    ncols = len(cons) * U
                    blocks.append(dict(kind="j4m", rp=rp, mp=mp, col0=col,
                                       ncols=ncols, ow=ow, cons=tuple(cons)))
                    col += ncols
        if mp % 2 == 1 and mp <= OW - 1:
            gp = (mp - 1) // 2               # covers ow = 2*gp, 2*gp+1
            for q in range(NO):
                blocks.append(dict(kind="j4c", q=q, mp=mp, col0=col,
                                   ncols=U, gp=gp))
                col += U
        pc = sum(bl["ncols"] for bl in blocks)
        wmax = max(wmax, pc)
        phases.append(blocks)
    return phases, col, wmax


W_SCALE = 32.0  # w stream is stored as float8_e3m4 * 32; host divides out


def pack_inputs(x, w, b):
    """Build the per-core input arrays. Returns list of dicts for in_maps."""
    import ml_dtypes

    x = np.ascontiguousarray(np.asarray(x, dtype=np.float32))
    w = np.asarray(w, dtype=np.float32)
    b = np.asarray(b, dtype=np.float32)

    phases, totcols, _ = build_schedule()

    # x: pad rows to HP, transpose to [h, w, c, b] fp16
    xT = np.zeros((HP, W, C, B), dtype=np.float16)
    xT[:H] = x.transpose(1, 2, 3, 0).astype(np.float16)

    # w: [OH,OW,U,K,K,C] -> wt [OHP, OW, K(i), K(j), C, U] fp32, padded oh rows
    wt = np.zeros((OHP, OW, K, K, C, U), dtype=np.float32)
    wt[:OH] = w.transpose(0, 1, 3, 4, 5, 2)

    # bias carries the W_SCALE so psum accumulates W_SCALE*(conv+bias);
    # the host gather divides it back out.
    bias_pad = np.zeros((OHP, OW, U), dtype=np.float32)
    bias_pad[:OH] = b * W_SCALE

    in_maps = []
    for core in range(NCORES):
        oh0 = core * NO
        r0 = 2 * oh0
        # x tile: [128, NMP*NR*B]; free = (mp, r, b).
        # Partition halves are PARITY-SWAPPED: tile mp holds its even column
        # (2mp) in partitions 0:64 when mp is even, in partitions 64:128 when
        # mp is odd (legacy layout; main blocks only need the per-pair
        # (dj, c) order, which the flip in the w pack mirrors).
        xc = xT[r0:r0 + NR]                                  # [NR, W, C, B]
        xc = xc.reshape(NR, NMP, 2, C, B)                    # [r, mp, dj, c, b]
        xc = xc.transpose(1, 2, 3, 0, 4)                     # [mp, dj, c, r, b]
        xc = xc.copy()
        xc[1::2] = xc[1::2, ::-1]                            # swap halves, odd mp
        xtile = np.ascontiguousarray(
            xc.transpose(1, 2, 0, 3, 4).reshape(128, NMP * NR * B))

        # x4a: [128=(di,c), OW*5*B]; free = (ow, rp, b); row 2*rp+di of this
        # core, column 2*ow+4. Serves the K=128 j4m blocks (j=4, i=2*?+di).
        rows = xT[r0:r0 + 10].reshape(NO + 1, 2, W, C, B)    # [rp, di, w, c, b]
        cols4 = rows[:, :, 4:4 + 2 * OW:2]                   # [rp, di, ow, c, b]
        x4a = np.ascontiguousarray(
            cols4.transpose(1, 3, 2, 0, 4).reshape(128, OW * (NO + 1) * B))

        # x4b: [128=(dg,c), 15*NO*B]; free = (gp, q, b); row 2*q+4, column
        # 4*gp+4+2*dg. Serves the K=64 corner-tap (i=4, j=4) matmuls.
        rowsb = xT[r0 + 4:r0 + 4 + 2 * NO:2]                 # [q, w, c, b]
        colsb = rowsb[:, 4::2][:, :2 * (OW // 2)]            # [q, g, c, b] g: col 4+2g
        colsb = colsb.reshape(NO, OW // 2, 2, C, B)          # [q, gp, dg, c, b]
        x4b = np.ascontiguousarray(
            colsb.transpose(2, 3, 1, 0, 4).reshape(128, (OW // 2) * NO * B))

        # w stream (built fp32, quantized to e3m4 at the end)
        ws = np.zeros((128, totcols), dtype=np.float32)
        for mp, blocks in enumerate(phases):
            flip = (mp % 2 == 1)
            for bl in blocks:
                c0 = bl["col0"]
                if bl["kind"] in ("main", "main2"):
                    lo, noh = bl["oh0"], bl["noh"]
                    ohs = np.arange(lo, lo + noh)
                    i_s = bl["r"] - 2 * ohs
                    if bl["kind"] == "main2":
                        parts = [(mp - 1, 2), (mp, 0)]   # (ow, j0), dow order
                    else:
                        parts = [(bl["ow"], 0 if bl["role"] == 0 else 2)]
                    npart = len(parts)
                    for k, (ow, j0) in enumerate(parts):
                        # [noh, 2(dj), C, U]
                        src = wt[oh0 + ohs, ow, i_s, j0:j0 + 2]
                        if flip:
                            src = src[:, ::-1]
                        # -> [128=(dj,c), noh, U]; cols (oh, dow, u)-ordered
                        blk = src.transpose(1, 2, 0, 3).reshape(128, noh, U)
                        for t in range(noh):
                            cc = c0 + (t * npart + k) * U
                            ws[:, cc:cc + U] = blk[:, t]
                elif bl["kind"] == "j4m":
                    ow = bl["ow"]
                    for k, (oh, ib) in enumerate(bl["cons"]):
                        for di in range(2):
                            ws[di * C:(di + 1) * C,
                               c0 + k * U:c0 + (k + 1) * U] = \
                                wt[oh0 + oh, ow, ib + di, 4]
                else:  # j4c
                    gp, q = bl["gp"], bl["q"]
                    ws[0:C, c0:c0 + U] = wt[oh0 + q, 2 * gp, 4, 4]
                    ws[C:2 * C, c0:c0 + U] = wt[oh0 + q, 2 * gp + 1, 4, 4]

        # bias: [1, OW*NO*U] fp32, (ow, oh_l, u) order; broadcast on device
        bias_1 = np.ascontiguousarray(
            bias_pad[oh0:oh0 + NO].transpose(1, 0, 2).reshape(1, OW * NO * U))

        ws8 = (ws * W_SCALE).astype(ml_dtypes.float8_e3m4)
        in_maps.append({"xt": xtile, "x4a": x4a, "x4b": x4b,
                        "wstream": ws8, "bias_1": bias_1})
    return in_maps


def emulate_core(inp):
    """Numpy emulation of the device program for one core (validation)."""
    phases, totcols, _ = build_schedule()
    xt = inp["xt"].astype(np.float32)
    x4a = inp["x4a"].astype(np.float32)
    x4b = inp["x4b"].astype(np.float32)
    ws = inp["wstream"].astype(np.float32)
    bias = np.broadcast_to(inp["bias_1"], (64, OW * NO * U))
    pts = {}
    stag = np.zeros((64, OW, NO, U), dtype=np.float32)
    for mp, blocks in enumerate(phases):
        if mp <= OW - 1:
            pts[mp] = np.zeros((64, NO, U), dtype=np.float32)
        for bl in blocks:
            rhs = ws[:, bl["col0"]:bl["col0"] + bl["ncols"]]
            if bl["kind"] == "main":
                lo, noh = bl["oh0"], bl["noh"]
                xoff = (mp * NR + bl["r"]) * B
                lhsT = xt[:, xoff:xoff + B]
                res = lhsT.T @ rhs                       # [64, noh*64]
                pts[bl["ow"]][:, lo:lo + noh, :] += res.reshape(64, noh, U)
            elif bl["kind"] == "main2":
                lo, noh = bl["oh0"], bl["noh"]
                xoff = (mp * NR + bl["r"]) * B
                lhsT = xt[:, xoff:xoff + B]
                res = (lhsT.T @ rhs).reshape(64, noh, 2, U)
                pts[mp - 1][:, lo:lo + noh, :] += res[:, :, 0]
                pts[mp][:, lo:lo + noh, :] += res[:, :, 1]
            elif bl["kind"] == "j4m":
                ow, rp = bl["ow"], bl["rp"]
                xoff = (ow * (NO + 1) + rp) * B
                lhsT = x4a[:, xoff:xoff + B]
                res = lhsT.T @ rhs                       # [64, ncons*64]
                oh_lo = bl["cons"][0][0]
                nc_ = len(bl["cons"])
                pts[ow][:, oh_lo:oh_lo + nc_, :] += res.reshape(64, nc_, U)
            else:  # j4c
                gp, q = bl["gp"], bl["q"]
                xoff = (gp * NO + q) * B
                for dg in range(2):
                    lhsT = x4b[dg * C:(dg + 1) * C, xoff:xoff + B]
                    res = lhsT.T @ rhs[dg * C:(dg + 1) * C]
                    pts[2 * gp + dg][:, q, :] += res
        ow = mp - 2
        if 0 <= ow <= OW - 1:
            bv = bias[:, ow * NO * U:(ow + 1) * NO * U].reshape(64, NO, U)
            stag[:, ow] = pts.pop(ow) + bv
    return stag / W_SCALE  # [64, ow, oh_l, u]


# ---------------- device kernel ----------------

def build_nc(loop_n=1):
    """Build the device program. loop_n > 1 wraps the whole phase sweep in a
    hardware For_i loop (identical work each iteration) — used only to
    measure per-iteration HW time above the RPC noise floor."""
    import concourse.bass as bass  # noqa: F401
    import concourse.mybir as mybir
    import concourse.tile as tile
    from concourse import bacc

    phases, totcols, wmax = build_schedule()
    dt = mybir.dt

    ablate = os.environ.get("KABLATE", "")  # dev-only: "nomm","nodve","nodma"
    nc = bacc.Bacc("TRN2", target_bir_lowering=False, debug=False,
                   num_devices=NCORES)
    xt_d = nc.dram_tensor("xt", [128, NMP * NR * B], dt.float16,
                          kind="ExternalInput").ap()
    x4a_d = nc.dram_tensor("x4a", [128, OW * (NO + 1) * B], dt.float16,
                           kind="ExternalInput").ap()
    x4b_d = nc.dram_tensor("x4b", [128, (OW // 2) * NO * B], dt.float16,
                           kind="ExternalInput").ap()
    ws_d = nc.dram_tensor("wstream", [128, totcols], dt.float8e3,
                          kind="ExternalInput").ap()
    bias_d = nc.dram_tensor("bias_1", [1, OW * NO * U], dt.float32,
                            kind="ExternalInput").ap()
    out_d = nc.dram_tensor("out", [B, OW, NO, U], dt.bfloat16,
                           kind="ExternalOutput").ap()

    with tile.TileContext(nc) as tc:
        with tc.tile_pool(name="xpool", bufs=1) as xpool, \
             tc.tile_pool(name="bpool", bufs=1) as bpool, \
             tc.tile_pool(name="stpool", bufs=1) as stpool, \
             tc.tile_pool(name="wpool", bufs=int(os.environ.get("WBUFS","8"))) as wpool, \
             tc.tile_pool(name="tmppool", bufs=4) as tmppool, \
             tc.tile_pool(name="pspool", bufs=int(os.environ.get("PSBUFS","5")), space="PSUM") as pspool:

            # Two HWDGE rings: w phase DMAs alternate between them; the x
            # preload is chunked on the ACT ring so early w phases can start
            # while later x chunks stream in.
            dma_w = nc.sync
            dma_x = nc.scalar

            xsb = xpool.tile([128, NMP * NR * B], dt.float16, tag="xt")
            XCH = int(os.environ.get("XCH", "4"))  # x chunks
            xch = NMP // XCH * NR * B
            for g in range(XCH):
                dma_x.dma_start(xsb[:, g * xch:(g + 1) * xch],
                                xt_d[:, g * xch:(g + 1) * xch])
            x4asb = xpool.tile([128, OW * (NO + 1) * B], dt.float16,
                               tag="x4a")
            dma_x.dma_start(x4asb[:, :], x4a_d[:, :])
            x4bsb = xpool.tile([128, (OW // 2) * NO * B], dt.float16,
                               tag="x4b")
            dma_x.dma_start(x4bsb[:, :], x4b_d[:, :])
            b1 = bpool.tile([1, OW * NO * U], dt.float32, tag="b1")
            dma_x.dma_start(b1[:, :], bias_d[:, :])
            bsb = bpool.tile([64, OW * NO * U], dt.float32, tag="brep")
            nc.gpsimd.partition_broadcast(bsb[:, :], b1[:, :], channels=64)
            stag = stpool.tile([64, OW * NO * U], dt.bfloat16)

            import contextlib
            loop_cm = (tc.For_i(0, loop_n, 1) if loop_n > 1
                       else contextlib.nullcontext())
            with loop_cm:
                _emit_sweep(nc, tc, phases, wmax, dt, ablate, dma_w, dma_x,
                            xsb, x4asb, x4bsb, bsb, stag, ws_d, out_d,
                            wpool, tmppool, pspool)

    nc.compile()
    return nc


def _emit_sweep(nc, tc, phases, wmax, dt, ablate, dma_w, dma_x,
                xsb, x4asb, x4bsb, bsb, stag, ws_d, out_d,
                wpool, tmppool, pspool):
    pts = {}
    for mp, blocks in enumerate(phases):
        wcols = sum(bl["ncols"] for bl in blocks)
        if wcols:
            pc0 = blocks[0]["col0"]
            wsb = wpool.tile([128, wmax], dt.float8e3, tag="wstream")
            if ablate != "nodma":
                ring = dma_w if mp % 2 == 0 else dma_x
                ring.dma_start(wsb[:, :wcols],
                               ws_d[:, pc0:pc0 + wcols])

        # PSUM tiles. KPAIR=0: per-ow [64,256] (v5); KPAIR=1: ow-pair
        # [64,512] one bank, interleaved (oh, dow, u) so merged matmuls
        # write a contiguous range; KPAIR=2: ow-pair, dow-major (contiguous
        # per-ow halves, no merge support).
        kpair = os.environ.get("KPAIR", "1")
        if kpair == "0":
            if mp <= OW - 1:
                pt = pspool.tile([64, NO * U], dt.float32)
                pts[mp] = pt
                if ablate not in ("nodve", "mmonly"):
                    nc.vector.memset(pt[:, :], 0.0)
        else:
            if mp % 2 == 0 and mp <= OW - 2:
                pt = pspool.tile([64, 2 * NO * U], dt.float32)
                pts[mp // 2] = pt
                if ablate not in ("nodve", "mmonly"):
                    nc.vector.memset(pt[:, :], 0.0)

        def _owv(ow):
            if kpair == "0":
                return pts[ow][:, :].rearrange("p (o u) -> p o u", o=NO, u=U)
            if kpair == "2":
                return pts[ow // 2][:, (ow % 2) * NO * U:
                                    (ow % 2 + 1) * NO * U].rearrange(
                    "p (o u) -> p o u", o=NO, u=U)
            return pts[ow // 2][:, :].rearrange(
                "p (o d u) -> p o d u", o=NO, d=2, u=U)[:, :, ow % 2]

        for bl in blocks:
            loc0 = bl["col0"] - pc0
            rhs = wsb[:, loc0:loc0 + bl["ncols"]]
            if bl["kind"] == "main":
                lo, noh = bl["oh0"], bl["noh"]
                xoff = (mp * NR + bl["r"]) * B
                lhsT = xsb[:, xoff:xoff + B]
                outap = _owv(bl["ow"])[:, lo:lo + noh, :]
                if ablate != "nomm":
                    nc.tensor.matmul(outap, lhsT, rhs, start=False,
                                     stop=False, skip_group_check=True)
            elif bl["kind"] == "main2":
                lo, noh = bl["oh0"], bl["noh"]
                xoff = (mp * NR + bl["r"]) * B
                lhsT = xsb[:, xoff:xoff + B]
                outap = pts[(mp - 1) // 2][
                    :, lo * 2 * U:(lo + noh) * 2 * U]
                if ablate != "nomm":
                    nc.tensor.matmul(outap, lhsT, rhs, start=False,
                                     stop=False, skip_group_check=True)
            elif bl["kind"] == "j4m":
                ow, rp = bl["ow"], bl["rp"]
                xoff = (ow * (NO + 1) + rp) * B
                lhsT = x4asb[:, xoff:xoff + B]
                oh_lo = bl["cons"][0][0]
                outap = _owv(ow)[:, oh_lo:oh_lo + len(bl["cons"]), :]
                if ablate != "nomm":
                    nc.tensor.matmul(outap, lhsT, rhs, start=False,
                                     stop=False, skip_group_check=True)
            else:  # j4c: corner tap, two K=64 matmuls (ow = 2gp, 2gp+1)
                gp, q = bl["gp"], bl["q"]
                xoff = (gp * NO + q) * B
                for dg in range(2):
                    lhsT = x4bsb[dg * C:(dg + 1) * C, xoff:xoff + B]
                    rhs4 = wsb[dg * C:(dg + 1) * C,
                               loc0:loc0 + U]
                    outap = _owv(2 * gp + dg)[:, q:q + 1, :]
                    if ablate != "nomm":
                        nc.tensor.matmul(outap, lhsT, rhs4, start=False,
                                         stop=False,
                                         skip_group_check=True)

        ow = mp - 2
        if 0 <= ow <= OW - 1:
            a1 = _owv(ow)
            bv = bsb[:, ow * NO * U:(ow + 1) * NO * U].rearrange(
                "p (o u) -> p o u", u=U)
            stv = stag[:, ow * NO * U:(ow + 1) * NO * U].rearrange(
                "p (o u) -> p o u", u=U)
            if ablate not in ("nodve", "mmonly") or (
                    ablate == "mmonly" and ow == OW - 1):
                nc.vector.tensor_add(stv, a1, bv)
            if kpair == "0":
                del pts[ow]
            elif ow % 2 == 1:
                del pts[ow // 2]
            if ablate == "mmonly":
                if ow == OW - 1:  # keep the output written for the verifier
                    sl = slice(ow * NO * U, (ow + 1) * NO * U)
                    dma_w.dma_start(
                        out_d.rearrange("b w o u -> b (w o u)")[:, sl],
                        stag[:, sl])
                continue
            # stream the output out as rows complete: 8-ow chunks early,
            # then 2-ow chunks so the tail DMAs overlap the final drains
            if ow < 24 and ow % 8 == 7:
                g = ow // 8
                sl = slice(g * 8 * NO * U, (g + 1) * 8 * NO * U)
                dma_w.dma_start(
                    out_d.rearrange("b w o u -> b (w o u)")[:, sl],
                    stag[:, sl])
            elif ow >= 24 and ow % 2 == 1:
                sl = slice((ow - 1) * NO * U, (ow + 1) * NO * U)
                dma_w.dma_start(
                    out_d.rearrange("b w o u -> b (w o u)")[:, sl],
                    stag[:, sl])


def _exec(nc, in_maps, repeats=1, chain=1):
    """Execute the prebuilt Bass module on the 8 cores via PJRT/axon.

    Mirrors bass2jax.run_bass_via_pjrt's multi-core branch, but keeps the
    jitted executable + device-staged inputs so the kernel can be re-run for
    timing. `chain` repeats the kernel execution inside one program (for
    amortized on-device timing). Returns (per_core_results, wall_times_s).
    """
    import time

    import jax
    import numpy as _np
    from jax.sharding import Mesh, NamedSharding, PartitionSpec

    try:
        from jax.experimental.shard_map import shard_map
    except ImportError:
        from jax.shard_map import shard_map

    import concourse.mybir as mybir
    from concourse import bass2jax

    bass2jax.install_neuronx_cc_hook()

    partition_name = (nc.partition_id_tensor.name
                      if nc.partition_id_tensor else None)
    in_names, out_names, out_avals, zero_outs = [], [], [], []
    for alloc in nc.m.functions[0].allocations:
        if not isinstance(alloc, mybir.MemoryLocationSet):
            continue
        name = alloc.memorylocations[0].name
        if alloc.kind == "ExternalInput":
            if name != partition_name:
                in_names.append(name)
        elif alloc.kind == "ExternalOutput":
            out_names.append(name)
            shape = tuple(alloc.tensor_shape)
            dtype = mybir.dt.np(alloc.dtype)
            out_avals.append(jax.core.ShapedArray(shape, dtype))
            zero_outs.append(_np.zeros(shape, dtype))
    n_params = len(in_names)
    all_names = in_names + out_names
    if partition_name is not None:
        all_names = all_names + [partition_name]

    def _bind(operands):
        return bass2jax._bass_exec_p.bind(
            *operands,
            out_avals=tuple(out_avals),
            in_names=tuple(all_names),
            out_names=tuple(out_names),
            lowering_input_output_aliases=(),
            sim_require_finite=True,
            sim_require_nnan=True,
            nc=nc,
        )

    def _body(*args):
        operands = list(args)
        if partition_name is not None:
            operands.append(bass2jax.partition_id_tensor())
        return tuple(_bind(operands))

    n_cores = len(in_maps)
    devices = jax.devices()[:n_cores]
    mesh = Mesh(_np.asarray(devices), ("core",))
    spec = PartitionSpec("core")
    sharded = jax.jit(
        shard_map(_body, mesh=mesh, in_specs=(spec,) * (n_params + len(out_names)),
                  out_specs=(spec,) * len(out_names), check_rep=False),
        keep_unused=True,
    )
    sharding = NamedSharding(mesh, spec)
    staged = [
        jax.device_put(
            _np.concatenate([_np.asarray(m[name]) for m in in_maps], axis=0),
            sharding)
        for name in in_names
    ] + [
        jax.device_put(
            _np.zeros((n_cores * z.shape[0], *z.shape[1:]), z.dtype), sharding)
        for z in zero_outs
    ]

    times = []
    out_arrs = None
    for _ in range(max(1, repeats)):
        t0 = time.perf_counter()
        out_arrs = jax.block_until_ready(sharded(*staged))
        times.append(time.perf_counter() - t0)

    results = [
        {
            name: _np.asarray(out_arrs[i]).reshape(n_cores, *out_avals[i].shape)[c]
            for i, name in enumerate(out_names)
        }
        for c in range(n_cores)
    ]
    return results, times


def _run(inputs, repeats=1):
    """Run on hardware. Returns (full_output, wall_times_s)."""
    in_maps = pack_inputs(inputs["x"], inputs["w"], inputs["b"])
    nc = build_nc()
    results, times = _exec(nc, in_maps, repeats=repeats)
    return _gather(results), times


def _gather(results):
    out = np.empty((B, OHP, OW, U), dtype=np.float32)
    for c in range(NCORES):
        # per-core out [B, OW, NO, U] bf16 (scaled by W_SCALE)
        out[:, c * NO:(c + 1) * NO] = (
            results[c]["out"].astype(np.float32).transpose(0, 2, 1, 3))
    return out[:, :OH] * (1.0 / W_SCALE)


def kernel(x, w, b):
    from concourse.bass_utils import run_bass_kernel_spmd

    in_maps = pack_inputs(x, w, b)
    nc = build_nc()
    res = run_bass_kernel_spmd(nc, in_maps, list(range(NCORES)))
    return _gather(res.results)



# revision 50
# speedup vs baseline: 1456.4219x; 1456.4219x over previous
"""FreeConv2D (locally-connected conv2d + bias) Trainium2 Bass kernel.

out[b,oh,ow,u] = sum_{i,j,c} w[oh,ow,u,i,j,c] * x[b, oh*2+i, ow*2+j, c] + bias[oh,ow,u]

Shapes: x [64,64,64,64], w [30,30,64,5,5,64], b [30,30,64] -> out [64,30,30,64].

Strategy (8 NeuronCores):
  - Shard output rows OH over cores: 4 rows/core (padded 30->32; last 2 dummy).
  - The kernel is DMA-bound (~330 GB/s/core aggregate): w dominates traffic,
    so the w stream is stored as float8_e3m4 * 32 (half the bytes of fp16;
    measured rel err ~1.1e-2 vs the 2e-2 gate) and the output as bf16. The
    matmul mixes lhsT fp16 (x) with rhs fp8e3 (w) — allowed on TRN2.
  - PSUM tiles are not memset: the first matmul into each (oh, role) slot
    uses start=True (even-r blocks are split so the fresh-oh part is its own
    matmul), which keeps the DVE free for drains.
  - Host pre-packs (numpy, not counted in HW time):
      * x    -> per-core fp16 tile [128, 11*32*64]: partition p = dj*64+c for
               column pair (2*mp, 2*mp+1), free = (r, mp, b).
      * w    -> per-core fp8e3 stream [128, TOT] (values * 32): matmul rhs
               blocks in execution order (column-pair taps j in {0,1} / {2,3}
               as K=128 blocks; j=4 taps as K=64 vertically-paired blocks).
      * bias -> per-core fp32 [64, 30*4*64] * 32 replicated over batch
               partitions; host gather divides the 32 back out.
  - Device: 32-phase sweep over column pairs mp. Phase mp:
      * DMA the phase's w blocks (~1 MB).
      * psum phase tile pt[mp] [64, 512] = accum slots (oh_l, role) where
        role 0 = j01-half of loc (oh, mp), role 1 = j23-half of loc (oh, mp-1).
      * matmuls: lhsT = resident x tile [128, 64(b)] (stationary),
        rhs = w blocks [128, N<=512] (moving), accumulate with start=False
        (tiles pre-zeroed by DVE memset; psum has_written semantics make this
        correct whether the first PE write accumulates or overwrites).
      * j=4 taps (K=64) of loc (oh, mp-2) also land in pt[mp-2] role-0 slots.
      * drain loc (.., ow=mp-2): out = pt[ow].role0 + bias + pt[ow+1].role1
        via two DVE tensor_adds into an SBUF staging buffer.
  - One final DMA of staging -> DRAM out [64, 30(ow), 4(oh_l), 64] per core;
    host gathers/transposes/trims to [64, 30, 30, 64].
"""

import os
import sys

import numpy as np

_TRN_REPO = "/opt/trn_rl_repo"
if _TRN_REPO not in sys.path:
    sys.path.insert(0, _TRN_REPO)

# The kernel needs the axon/neuron jax backend; a JAX_PLATFORMS=cpu pin (used
# for reference computation) would hide the NeuronCores. Only effective if jax
# has not been initialized yet in this process.
if "jax" not in sys.modules and "axon" not in os.environ.get("JAX_PLATFORMS", "axon"):
    os.environ.pop("JAX_PLATFORMS", None)

# ---------------- problem constants (hardcoded) ----------------
B, H, W, C = 64, 64, 64, 64
U, K, S = 64, 5, 2
OH = OW = 30
NCORES = 8
NO = 4                      # oh rows per core (padded: 8*4 = 32 >= 30)
OHP = NCORES * NO           # 32
NR = 2 * (NO - 1) + K       # 11 input rows per core
NMP = 32                    # column-pair tiles mp=0..31; also phase count
NT4 = OW // 2               # 15 j4 ow-pairs
HP = 2 * (OHP - 1) + K      # 67 padded input rows overall


def _oh_span(r):
    """Valid local oh range for local input row r: i = r - 2*oh in [0, K-1]."""
    lo = max(0, -(-(r - (K - 1)) // 2))   # ceil((r-4)/2)
    hi = min(NO - 1, r // 2)
    return lo, hi


def build_schedule():
    """Per-phase block lists. Block cols are offsets into the packed w stream.

    Accumulation is single-slot: each output column ow owns one PSUM tile
    PS[ow] [64, NO*U]; every matmul targets the owning tile directly.

    Returns (phases, totcols, wmax) where phases[mp] is a list of dicts:
      kind 'main': K=128 block; role 0 = taps j=(0,1) for ow=mp,
                   role 1 = taps j=(2,3) for ow=mp-1 (separate blocks, each
                   targeting PS[ow]); ncols = noh*64.
      kind 'j4m':  K=128 block for tap j=4, row-pairs: partitions
                   (di, c) = input rows (2*rp+di); serves consumers
                   (oh=rp-1, i=2+di) and (oh=rp, i=di) of ow=mp-1, packed
                   oh-ascending in N; lhsT comes from the x4a tile.
      kind 'j4c':  corner tap (i=4, j=4), K=64, vertically paired in the
                   stream: rows 0:64 = w(oh=q, ow=2*gp), rows 64:128 =
                   w(oh=q, ow=2*gp+1); two matmuls (lhsT from x4b halves),
                   emitted on odd phases mp=2*gp+1; ncols = 64.
    """
    phases = []
    col = 0
    wmax = 0
    for mp in range(NMP):
        blocks = []
        for r in range(NR):
            if mp <= OW:  # main blocks exist for mp=0..30
                lo, hi = _oh_span(r)
                if lo > hi:
                    continue
                noh = hi - lo + 1
                roles = tuple(role for role in (0, 1)
                              if 0 <= mp - role <= OW - 1)
                # one matmul per (mp, r): cols (oh, role, u) interleaved to
                # match the 2-role psum tile layout (contiguous when both
                # roles are present)
                ncols = noh * len(roles) * U
                blocks.append(dict(kind="main", r=r, mp=mp, col0=col,
                                   ncols=ncols, oh0=lo, noh=noh,
                                   roles=roles))
                col += ncols
        if 1 <= mp <= OW:
            ow = mp - 1
            for rp in range(NO + 1):
                cons = []                    # (oh, i_base), oh ascending
                if 0 <= rp - 1 <= NO - 1:
                    cons.append((rp - 1, 2))
                if rp <= NO - 1:
                    cons.append((rp, 0))
                if cons:
                    ncols = len(cons) * U
                    blocks.append(dict(kind="j4m", rp=rp, mp=mp, col0=col,
                                       ncols=ncols, ow=ow, cons=tuple(cons)))
                    col += ncols
        if mp % 2 == 1 and mp <= OW - 1:
            gp = (mp - 1) // 2               # covers ow = 2*gp, 2*gp+1
            for q in range(NO):
                blocks.append(dict(kind="j4c", q=q, mp=mp, col0=col,
                                   ncols=U, gp=gp))
                col += U
        pc = sum(bl["ncols"] for bl in blocks)
        wmax = max(wmax, pc)
        phases.append(blocks)
    return phases, col, wmax


W_SCALE = 32.0  # w stream is stored as float8_e3m4 * 32; host divides out


def pack_inputs(x, w, b):
    """Build the per-core input arrays. Returns list of dicts for in_maps."""
    import ml_dtypes

    x = np.ascontiguousarray(np.asarray(x, dtype=np.float32))
    w = np.asarray(w, dtype=np.float32)
    b = np.asarray(b, dtype=np.float32)

    phases, totcols, _ = build_schedule()

    # x: pad rows to HP, transpose to [h, w, c, b] fp16
    xT = np.zeros((HP, W, C, B), dtype=np.float16)
    xT[:H] = x.transpose(1, 2, 3, 0).astype(np.float16)

    # w: [OH,OW,U,K,K,C] -> wt [OHP, OW, K(i), K(j), C, U] fp32, padded oh rows
    wt = np.zeros((OHP, OW, K, K, C, U), dtype=np.float32)
    wt[:OH] = w.transpose(0, 1, 3, 4, 5, 2)

    # bias carries the W_SCALE so psum accumulates W_SCALE*(conv+bias);
    # the host gather divides it back out.
    bias_pad = np.zeros((OHP, OW, U), dtype=np.float32)
    bias_pad[:OH] = b * W_SCALE

    in_maps = []
    for core in range(NCORES):
        oh0 = core * NO
        r0 = 2 * oh0
        # x tile: [128, NMP*NR*B]; free = (mp, r, b).
        # Partition halves are PARITY-SWAPPED: tile mp holds its even column
        # (2mp) in partitions 0:64 when mp is even, in partitions 64:128 when
        # mp is odd (legacy layout; main blocks only need the per-pair
        # (dj, c) order, which the flip in the w pack mirrors).
        xc = xT[r0:r0 + NR]                                  # [NR, W, C, B]
        xc = xc.reshape(NR, NMP, 2, C, B)                    # [r, mp, dj, c, b]
        xc = xc.transpose(1, 2, 3, 0, 4)                     # [mp, dj, c, r, b]
        xc = xc.copy()
        xc[1::2] = xc[1::2, ::-1]                            # swap halves, odd mp
        xtile = np.ascontiguousarray(
            xc.transpose(1, 2, 0, 3, 4).reshape(128, NMP * NR * B))

        # x4a: [128=(di,c), OW*5*B]; free = (ow, rp, b); row 2*rp+di of this
        # core, column 2*ow+4. Serves the K=128 j4m blocks (j=4, i=2*?+di).
        rows = xT[r0:r0 + 10].reshape(NO + 1, 2, W, C, B)    # [rp, di, w, c, b]
        cols4 = rows[:, :, 4:4 + 2 * OW:2]                   # [rp, di, ow, c, b]
        x4a = np.ascontiguousarray(
            cols4.transpose(1, 3, 2, 0, 4).reshape(128, OW * (NO + 1) * B))

        # x4b: [128=(dg,c), 15*NO*B]; free = (gp, q, b); row 2*q+4, column
        # 4*gp+4+2*dg. Serves the K=64 corner-tap (i=4, j=4) matmuls.
        rowsb = xT[r0 + 4:r0 + 4 + 2 * NO:2]                 # [q, w, c, b]
        colsb = rowsb[:, 4::2][:, :2 * (OW // 2)]            # [q, g, c, b] g: col 4+2g
        colsb = colsb.reshape(NO, OW // 2, 2, C, B)          # [q, gp, dg, c, b]
        x4b = np.ascontiguousarray(
            colsb.transpose(2, 3, 1, 0, 4).reshape(128, (OW // 2) * NO * B))

        # w stream (built fp32, quantized to e3m4 at the end)
        ws = np.zeros((128, totcols), dtype=np.float32)
        for mp, blocks in enumerate(phases):
            flip = (mp % 2 == 1)
            for bl in blocks:
                c0 = bl["col0"]
                if bl["kind"] == "main":
                    lo, noh = bl["oh0"], bl["noh"]
                    ohs = np.arange(lo, lo + noh)
                    i_s = bl["r"] - 2 * ohs
                    nroles = len(bl["roles"])
                    for k, role in enumerate(bl["roles"]):
                        ow, j0 = mp - role, 2 * role
                        # [noh, 2(dj), C, U]
                        src = wt[oh0 + ohs, ow, i_s, j0:j0 + 2]
                        if flip:
                            src = src[:, ::-1]
                        # -> [128=(dj,c), noh, U]; cols (oh, role, u)-ordered
                        blk = src.transpose(1, 2, 0, 3).reshape(128, noh, U)
                        for t in range(noh):
                            cc = c0 + (t * nroles + k) * U
                            ws[:, cc:cc + U] = blk[:, t]
                elif bl["kind"] == "j4m":
                    ow = bl["ow"]
                    for k, (oh, ib) in enumerate(bl["cons"]):
                        for di in range(2):
                            ws[di * C:(di + 1) * C,
                               c0 + k * U:c0 + (k + 1) * U] = \
                                wt[oh0 + oh, ow, ib + di, 4]
                else:  # j4c
                    gp, q = bl["gp"], bl["q"]
                    ws[0:C, c0:c0 + U] = wt[oh0 + q, 2 * gp, 4, 4]
                    ws[C:2 * C, c0:c0 + U] = wt[oh0 + q, 2 * gp + 1, 4, 4]

        # bias: [1, OW*NO*U] fp32, (ow, oh_l, u) order; broadcast on device
        bias_1 = np.ascontiguousarray(
            bias_pad[oh0:oh0 + NO].transpose(1, 0, 2).reshape(1, OW * NO * U))

        ws8 = (ws * W_SCALE).astype(ml_dtypes.float8_e3m4)
        in_maps.append({"xt": xtile, "x4a": x4a, "x4b": x4b,
                        "wstream": ws8, "bias_1": bias_1})
    return in_maps


def emulate_core(inp):
    """Numpy emulation of the device program for one core (validation)."""
    phases, totcols, _ = build_schedule()
    xt = inp["xt"].astype(np.float32)
    x4a = inp["x4a"].astype(np.float32)
    x4b = inp["x4b"].astype(np.float32)
    ws = inp["wstream"].astype(np.float32)
    bias = np.broadcast_to(inp["bias_1"], (64, OW * NO * U))
    pts = {}
    stag = np.zeros((64, OW, NO, U), dtype=np.float32)
    for mp, blocks in enumerate(phases):
        if mp <= OW:
            pts[mp] = np.zeros((64, NO, 2, U), dtype=np.float32)
        for bl in blocks:
            rhs = ws[:, bl["col0"]:bl["col0"] + bl["ncols"]]
            if bl["kind"] == "main":
                lo, noh = bl["oh0"], bl["noh"]
                xoff = (mp * NR + bl["r"]) * B
                lhsT = xt[:, xoff:xoff + B]
                nroles = len(bl["roles"])
                res = (lhsT.T @ rhs).reshape(64, noh, nroles, U)
                for k, role in enumerate(bl["roles"]):
                    pts[mp][:, lo:lo + noh, role] += res[:, :, k]
            elif bl["kind"] == "j4m":
                ow, rp = bl["ow"], bl["rp"]
                xoff = (ow * (NO + 1) + rp) * B
                lhsT = x4a[:, xoff:xoff + B]
                res = lhsT.T @ rhs                       # [64, ncons*64]
                oh_lo = bl["cons"][0][0]
                nc_ = len(bl["cons"])
                pts[ow][:, oh_lo:oh_lo + nc_, 0] += res.reshape(64, nc_, U)
            else:  # j4c
                gp, q = bl["gp"], bl["q"]
                xoff = (gp * NO + q) * B
                for dg in range(2):
                    lhsT = x4b[dg * C:(dg + 1) * C, xoff:xoff + B]
                    res = lhsT.T @ rhs[dg * C:(dg + 1) * C]
                    pts[2 * gp + dg][:, q, 0] += res
        ow = mp - 2
        if 0 <= ow <= OW - 1:
            bv = bias[:, ow * NO * U:(ow + 1) * NO * U].reshape(64, NO, U)
            stag[:, ow] = pts.pop(ow)[:, :, 0] + pts[ow + 1][:, :, 1] + bv
    return stag / W_SCALE  # [64, ow, oh_l, u]


# ---------------- device kernel ----------------

def build_nc(loop_n=1):
    """Build the device program. loop_n > 1 wraps the whole phase sweep in a
    hardware For_i loop (identical work each iteration) — used only to
    measure per-iteration HW time above the RPC noise floor."""
    import concourse.bass as bass  # noqa: F401
    import concourse.mybir as mybir
    import concourse.tile as tile
    from concourse import bacc

    phases, totcols, wmax = build_schedule()
    dt = mybir.dt

    ablate = os.environ.get("KABLATE", "")  # dev-only: "nomm","nodve","nodma"
    nc = bacc.Bacc("TRN2", target_bir_lowering=False, debug=False,
                   num_devices=NCORES)
    xt_d = nc.dram_tensor("xt", [128, NMP * NR * B], dt.float16,
                          kind="ExternalInput").ap()
    x4a_d = nc.dram_tensor("x4a", [128, OW * (NO + 1) * B], dt.float16,
                           kind="ExternalInput").ap()
    x4b_d = nc.dram_tensor("x4b", [128, (OW // 2) * NO * B], dt.float16,
                           kind="ExternalInput").ap()
    ws_d = nc.dram_tensor("wstream", [128, totcols], dt.float8e3,
                          kind="ExternalInput").ap()
    bias_d = nc.dram_tensor("bias_1", [1, OW * NO * U], dt.float32,
                            kind="ExternalInput").ap()
    out_d = nc.dram_tensor("out", [B, OW, NO, U], dt.bfloat16,
                           kind="ExternalOutput").ap()

    with tile.TileContext(nc) as tc:
        with tc.tile_pool(name="xpool", bufs=1) as xpool, \
             tc.tile_pool(name="bpool", bufs=1) as bpool, \
             tc.tile_pool(name="stpool", bufs=1) as stpool, \
             tc.tile_pool(name="wpool", bufs=int(os.environ.get("WBUFS","8"))) as wpool, \
             tc.tile_pool(name="tmppool", bufs=4) as tmppool, \
             tc.tile_pool(name="pspool", bufs=int(os.environ.get("PSBUFS","5")), space="PSUM") as pspool:

            # Two HWDGE rings: w phase DMAs alternate between them; the x
            # preload is chunked on the ACT ring so early w phases can start
            # while later x chunks stream in.
            dma_w = nc.sync
            dma_x = nc.scalar

            xsb = xpool.tile([128, NMP * NR * B], dt.float16, tag="xt")
            XCH = int(os.environ.get("XCH", "4"))  # x chunks
            xch = NMP // XCH * NR * B
            for g in range(XCH):
                dma_x.dma_start(xsb[:, g * xch:(g + 1) * xch],
                                xt_d[:, g * xch:(g + 1) * xch])
            x4asb = xpool.tile([128, OW * (NO + 1) * B], dt.float16,
                               tag="x4a")
            dma_x.dma_start(x4asb[:, :], x4a_d[:, :])
            x4bsb = xpool.tile([128, (OW // 2) * NO * B], dt.float16,
                               tag="x4b")
            dma_x.dma_start(x4bsb[:, :], x4b_d[:, :])
            b1 = bpool.tile([1, OW * NO * U], dt.float32, tag="b1")
            dma_x.dma_start(b1[:, :], bias_d[:, :])
            bsb = bpool.tile([64, OW * NO * U], dt.float32, tag="brep")
            nc.gpsimd.partition_broadcast(bsb[:, :], b1[:, :], channels=64)
            zsb = bpool.tile([64, NO * 2 * U], dt.float32, tag="zeros")
            nc.vector.memset(zsb[:, :], 0.0)
            stag = stpool.tile([64, OW * NO * U], dt.bfloat16)

            import contextlib
            loop_cm = (tc.For_i(0, loop_n, 1) if loop_n > 1
                       else contextlib.nullcontext())
            with loop_cm:
                _emit_sweep(nc, tc, phases, wmax, dt, ablate, dma_w, dma_x,
                            xsb, x4asb, x4bsb, bsb, zsb, stag, ws_d, out_d,
                            wpool, tmppool, pspool)

    nc.compile()
    return nc


def _emit_sweep(nc, tc, phases, wmax, dt, ablate, dma_w, dma_x,
                xsb, x4asb, x4bsb, bsb, zsb, stag, ws_d, out_d,
                wpool, tmppool, pspool):
    pts = {}
    for mp, blocks in enumerate(phases):
        wcols = sum(bl["ncols"] for bl in blocks)
        if wcols:
            pc0 = blocks[0]["col0"]
            wsb = wpool.tile([128, wmax], dt.float8e3, tag="wstream")
            if ablate != "nodma":
                ring = dma_w if mp % 2 == 0 else dma_x
                ring.dma_start(wsb[:, :wcols],
                               ws_d[:, pc0:pc0 + wcols])

        # 2-role PSUM tile [64, 512] = one bank per phase (v1-proven):
        # free layout (oh, role, u); role0 = j01+j4 of loc (oh, mp),
        # role1 = j23 of loc (oh, mp-1). Zeroed on the Activation engine
        # to keep the DVE free for drains.
        if mp <= OW:
            pt = pspool.tile([64, NO * 2 * U], dt.float32)
            pts[mp] = pt
            if ablate != "mmonly":
                nc.scalar.copy(pt[:, :], zsb[:, :])

        def _rv(mp_, role):
            return pts[mp_][:, :].rearrange(
                "p (o r u) -> p o r u", o=NO, r=2, u=U)[:, :, role]

        for bl in blocks:
            loc0 = bl["col0"] - pc0
            rhs = wsb[:, loc0:loc0 + bl["ncols"]]
            if bl["kind"] == "main":
                lo, noh = bl["oh0"], bl["noh"]
                xoff = (mp * NR + bl["r"]) * B
                lhsT = xsb[:, xoff:xoff + B]
                if len(bl["roles"]) == 2:
                    # contiguous (oh, role, u) range
                    outap = pts[mp][:, lo * 2 * U:(lo + noh) * 2 * U]
                else:
                    outap = _rv(mp, bl["roles"][0])[:, lo:lo + noh, :]
                if ablate != "nomm":
                    nc.tensor.matmul(outap, lhsT, rhs, start=False,
                                     stop=False, skip_group_check=True)
            elif bl["kind"] == "j4m":
                ow, rp = bl["ow"], bl["rp"]
                xoff = (ow * (NO + 1) + rp) * B
                lhsT = x4asb[:, xoff:xoff + B]
                oh_lo = bl["cons"][0][0]
                outap = _rv(ow, 0)[:, oh_lo:oh_lo + len(bl["cons"]), :]
                if ablate != "nomm":
                    nc.tensor.matmul(outap, lhsT, rhs, start=False,
                                     stop=False, skip_group_check=True)
            else:  # j4c: corner tap, two K=64 matmuls (ow = 2gp, 2gp+1)
                gp, q = bl["gp"], bl["q"]
                xoff = (gp * NO + q) * B
                for dg in range(2):
                    lhsT = x4bsb[dg * C:(dg + 1) * C, xoff:xoff + B]
                    rhs4 = wsb[dg * C:(dg + 1) * C,
                               loc0:loc0 + U]
                    outap = _rv(2 * gp + dg, 0)[:, q:q + 1, :]
                    if ablate != "nomm":
                        nc.tensor.matmul(outap, lhsT, rhs4, start=False,
                                         stop=False,
                                         skip_group_check=True)

        ow = mp - 2
        if 0 <= ow <= OW - 1:
            a1 = _rv(ow, 0)
            a2 = _rv(ow + 1, 1)
            bv = bsb[:, ow * NO * U:(ow + 1) * NO * U].rearrange(
                "p (o u) -> p o u", u=U)
            stv = stag[:, ow * NO * U:(ow + 1) * NO * U].rearrange(
                "p (o u) -> p o u", u=U)
            if ablate not in ("nodve", "mmonly") or (
                    ablate == "mmonly" and ow == OW - 1):
                tmp = tmppool.tile([64, NO * U], dt.float32)
                tmpv = tmp[:, :].rearrange("p (o u) -> p o u", u=U)
                nc.vector.tensor_add(tmpv, a1, bv)
                nc.vector.tensor_add(stv, tmpv, a2)
            del pts[ow]
            if ablate == "mmonly":
                if ow == OW - 1:  # keep the output written for the verifier
                    sl = slice(ow * NO * U, (ow + 1) * NO * U)
                    dma_w.dma_start(
                        out_d.rearrange("b w o u -> b (w o u)")[:, sl],
                        stag[:, sl])
                continue
            # stream the output out as rows complete: 8-ow chunks early,
            # then 2-ow chunks so the tail DMAs overlap the final drains
            if ow < 24 and ow % 8 == 7:
                g = ow // 8
                sl = slice(g * 8 * NO * U, (g + 1) * 8 * NO * U)
                dma_w.dma_start(
                    out_d.rearrange("b w o u -> b (w o u)")[:, sl],
                    stag[:, sl])
            elif ow >= 24 and ow % 2 == 1:
                sl = slice((ow - 1) * NO * U, (ow + 1) * NO * U)
                dma_w.dma_start(
                    out_d.rearrange("b w o u -> b (w o u)")[:, sl],
                    stag[:, sl])


def _exec(nc, in_maps, repeats=1, chain=1):
    """Execute the prebuilt Bass module on the 8 cores via PJRT/axon.

    Mirrors bass2jax.run_bass_via_pjrt's multi-core branch, but keeps the
    jitted executable + device-staged inputs so the kernel can be re-run for
    timing. `chain` repeats the kernel execution inside one program (for
    amortized on-device timing). Returns (per_core_results, wall_times_s).
    """
    import time

    import jax
    import numpy as _np
    from jax.sharding import Mesh, NamedSharding, PartitionSpec

    try:
        from jax.experimental.shard_map import shard_map
    except ImportError:
        from jax.shard_map import shard_map

    import concourse.mybir as mybir
    from concourse import bass2jax

    bass2jax.install_neuronx_cc_hook()

    partition_name = (nc.partition_id_tensor.name
                      if nc.partition_id_tensor else None)
    in_names, out_names, out_avals, zero_outs = [], [], [], []
    for alloc in nc.m.functions[0].allocations:
        if not isinstance(alloc, mybir.MemoryLocationSet):
            continue
        name = alloc.memorylocations[0].name
        if alloc.kind == "ExternalInput":
            if name != partition_name:
                in_names.append(name)
        elif alloc.kind == "ExternalOutput":
            out_names.append(name)
            shape = tuple(alloc.tensor_shape)
            dtype = mybir.dt.np(alloc.dtype)
            out_avals.append(jax.core.ShapedArray(shape, dtype))
            zero_outs.append(_np.zeros(shape, dtype))
    n_params = len(in_names)
    all_names = in_names + out_names
    if partition_name is not None:
        all_names = all_names + [partition_name]

    def _bind(operands):
        return bass2jax._bass_exec_p.bind(
            *operands,
            out_avals=tuple(out_avals),
            in_names=tuple(all_names),
            out_names=tuple(out_names),
            lowering_input_output_aliases=(),
            sim_require_finite=True,
            sim_require_nnan=True,
            nc=nc,
        )

    def _body(*args):
        operands = list(args)
        if partition_name is not None:
            operands.append(bass2jax.partition_id_tensor())
        return tuple(_bind(operands))

    n_cores = len(in_maps)
    devices = jax.devices()[:n_cores]
    mesh = Mesh(_np.asarray(devices), ("core",))
    spec = PartitionSpec("core")
    sharded = jax.jit(
        shard_map(_body, mesh=mesh, in_specs=(spec,) * (n_params + len(out_names)),
                  out_specs=(spec,) * len(out_names), check_rep=False),
        keep_unused=True,
    )
    sharding = NamedSharding(mesh, spec)
    staged = [
        jax.device_put(
            _np.concatenate([_np.asarray(m[name]) for m in in_maps], axis=0),
            sharding)
        for name in in_names
    ] + [
        jax.device_put(
            _np.zeros((n_cores * z.shape[0], *z.shape[1:]), z.dtype), sharding)
        for z in zero_outs
    ]

    times = []
    out_arrs = None
    for _ in range(max(1, repeats)):
        t0 = time.perf_counter()
        out_arrs = jax.block_until_ready(sharded(*staged))
        times.append(time.perf_counter() - t0)

    results = [
        {
            name: _np.asarray(out_arrs[i]).reshape(n_cores, *out_avals[i].shape)[c]
            for i, name in enumerate(out_names)
        }
        for c in range(n_cores)
    ]
    return results, times


def _run(inputs, repeats=1):
    """Run on hardware. Returns (full_output, wall_times_s)."""
    in_maps = pack_inputs(inputs["x"], inputs["w"], inputs["b"])
    nc = build_nc()
    results, times = _exec(nc, in_maps, repeats=repeats)
    return _gather(results), times


def _gather(results):
    out = np.empty((B, OHP, OW, U), dtype=np.float32)
    for c in range(NCORES):
        # per-core out [B, OW, NO, U] bf16 (scaled by W_SCALE)
        out[:, c * NO:(c + 1) * NO] = (
            results[c]["out"].astype(np.float32).transpose(0, 2, 1, 3))
    return out[:, :OH] * (1.0 / W_SCALE)


def kernel(x, w, b):
    from concourse.bass_utils import run_bass_kernel_spmd

    in_maps = pack_inputs(x, w, b)
    nc = build_nc()
    res = run_bass_kernel_spmd(nc, in_maps, list(range(NCORES)))
    return _gather(res.results)



# revision 51
# speedup vs baseline: 1467.8297x; 1.0078x over previous
"""FreeConv2D (locally-connected conv2d + bias) Trainium2 Bass kernel.

out[b,oh,ow,u] = sum_{i,j,c} w[oh,ow,u,i,j,c] * x[b, oh*2+i, ow*2+j, c] + bias[oh,ow,u]

Shapes: x [64,64,64,64], w [30,30,64,5,5,64], b [30,30,64] -> out [64,30,30,64].

Strategy (8 NeuronCores):
  - Shard output rows OH over cores: 4 rows/core (padded 30->32; last 2 dummy).
  - The kernel is DMA-bound (~330 GB/s/core aggregate): w dominates traffic,
    so the w stream is stored as float8_e3m4 * 32 (half the bytes of fp16;
    measured rel err ~1.1e-2 vs the 2e-2 gate) and the output as bf16. The
    matmul mixes lhsT fp16 (x) with rhs fp8e3 (w) — allowed on TRN2.
  - PSUM tiles are not memset: the first matmul into each (oh, role) slot
    uses start=True (even-r blocks are split so the fresh-oh part is its own
    matmul), which keeps the DVE free for drains.
  - Host pre-packs (numpy, not counted in HW time):
      * x    -> per-core fp16 tile [128, 11*32*64]: partition p = dj*64+c for
               column pair (2*mp, 2*mp+1), free = (r, mp, b).
      * w    -> per-core fp8e3 stream [128, TOT] (values * 32): matmul rhs
               blocks in execution order (column-pair taps j in {0,1} / {2,3}
               as K=128 blocks; j=4 taps as K=64 vertically-paired blocks).
      * bias -> per-core fp32 [64, 30*4*64] * 32 replicated over batch
               partitions; host gather divides the 32 back out.
  - Device: 32-phase sweep over column pairs mp. Phase mp:
      * DMA the phase's w blocks (~1 MB).
      * psum phase tile pt[mp] [64, 512] = accum slots (oh_l, role) where
        role 0 = j01-half of loc (oh, mp), role 1 = j23-half of loc (oh, mp-1).
      * matmuls: lhsT = resident x tile [128, 64(b)] (stationary),
        rhs = w blocks [128, N<=512] (moving), accumulate with start=False
        (tiles pre-zeroed by DVE memset; psum has_written semantics make this
        correct whether the first PE write accumulates or overwrites).
      * j=4 taps (K=64) of loc (oh, mp-2) also land in pt[mp-2] role-0 slots.
      * drain loc (.., ow=mp-2): out = pt[ow].role0 + bias + pt[ow+1].role1
        via two DVE tensor_adds into an SBUF staging buffer.
  - One final DMA of staging -> DRAM out [64, 30(ow), 4(oh_l), 64] per core;
    host gathers/transposes/trims to [64, 30, 30, 64].
"""

import os
import sys

import numpy as np

_TRN_REPO = "/opt/trn_rl_repo"
if _TRN_REPO not in sys.path:
    sys.path.insert(0, _TRN_REPO)

# The kernel needs the axon/neuron jax backend; a JAX_PLATFORMS=cpu pin (used
# for reference computation) would hide the NeuronCores. Only effective if jax
# has not been initialized yet in this process.
if "jax" not in sys.modules and "axon" not in os.environ.get("JAX_PLATFORMS", "axon"):
    os.environ.pop("JAX_PLATFORMS", None)

# ---------------- problem constants (hardcoded) ----------------
B, H, W, C = 64, 64, 64, 64
U, K, S = 64, 5, 2
OH = OW = 30
NCORES = 8
NO = 4                      # oh rows per core (padded: 8*4 = 32 >= 30)
OHP = NCORES * NO           # 32
NR = 2 * (NO - 1) + K       # 11 input rows per core
NMP = 32                    # column-pair tiles mp=0..31; also phase count
NT4 = OW // 2               # 15 j4 ow-pairs
HP = 2 * (OHP - 1) + K      # 67 padded input rows overall


def _oh_span(r):
    """Valid local oh range for local input row r: i = r - 2*oh in [0, K-1]."""
    lo = max(0, -(-(r - (K - 1)) // 2))   # ceil((r-4)/2)
    hi = min(NO - 1, r // 2)
    return lo, hi


def build_schedule():
    """Per-phase block lists. Block cols are offsets into the packed w stream.

    Accumulation is single-slot: each output column ow owns one PSUM tile
    PS[ow] [64, NO*U]; every matmul targets the owning tile directly.

    Returns (phases, totcols, wmax) where phases[mp] is a list of dicts:
      kind 'main': K=128 block; role 0 = taps j=(0,1) for ow=mp,
                   role 1 = taps j=(2,3) for ow=mp-1 (separate blocks, each
                   targeting PS[ow]); ncols = noh*64.
      kind 'j4m':  K=128 block for tap j=4, row-pairs: partitions
                   (di, c) = input rows (2*rp+di); serves consumers
                   (oh=rp-1, i=2+di) and (oh=rp, i=di) of ow=mp-1, packed
                   oh-ascending in N; lhsT comes from the x4a tile.
      kind 'j4c':  corner tap (i=4, j=4), K=64, vertically paired in the
                   stream: rows 0:64 = w(oh=q, ow=2*gp), rows 64:128 =
                   w(oh=q, ow=2*gp+1); two matmuls (lhsT from x4b halves),
                   emitted on odd phases mp=2*gp+1; ncols = 64.
    """
    phases = []
    col = 0
    wmax = 0
    for mp in range(NMP):
        blocks = []
        for r in range(NR):
            if mp <= OW:  # main blocks exist for mp=0..30
                lo, hi = _oh_span(r)
                if lo > hi:
                    continue
                noh = hi - lo + 1
                roles = tuple(role for role in (0, 1)
                              if 0 <= mp - role <= OW - 1)
                # one matmul per (mp, r): cols (oh, role, u) interleaved to
                # match the 2-role psum tile layout (contiguous when both
                # roles are present)
                ncols = noh * len(roles) * U
                blocks.append(dict(kind="main", r=r, mp=mp, col0=col,
                                   ncols=ncols, oh0=lo, noh=noh,
                                   roles=roles))
                col += ncols
        if 1 <= mp <= OW:
            ow = mp - 1
            for rp in range(NO + 1):
                cons = []                    # (oh, i_base), oh ascending
                if 0 <= rp - 1 <= NO - 1:
                    cons.append((rp - 1, 2))
                if rp <= NO - 1:
                    cons.append((rp, 0))
                if cons:
                    ncols = len(cons) * U
                    blocks.append(dict(kind="j4m", rp=rp, mp=mp, col0=col,
                                       ncols=ncols, ow=ow, cons=tuple(cons)))
                    col += ncols
        if mp % 2 == 1 and mp <= OW - 1:
            gp = (mp - 1) // 2               # covers ow = 2*gp, 2*gp+1
            for q in range(NO):
                blocks.append(dict(kind="j4c", q=q, mp=mp, col0=col,
                                   ncols=U, gp=gp))
                col += U
        pc = sum(bl["ncols"] for bl in blocks)
        wmax = max(wmax, pc)
        phases.append(blocks)
    return phases, col, wmax


W_SCALE = 32.0  # w stream is stored as float8_e3m4 * 32; host divides out


def pack_inputs(x, w, b):
    """Build the per-core input arrays. Returns list of dicts for in_maps."""
    import ml_dtypes

    x = np.ascontiguousarray(np.asarray(x, dtype=np.float32))
    w = np.asarray(w, dtype=np.float32)
    b = np.asarray(b, dtype=np.float32)

    phases, totcols, _ = build_schedule()

    # x: pad rows to HP, transpose to [h, w, c, b] fp16
    xT = np.zeros((HP, W, C, B), dtype=np.float16)
    xT[:H] = x.transpose(1, 2, 3, 0).astype(np.float16)

    # w: [OH,OW,U,K,K,C] -> wt [OHP, OW, K(i), K(j), C, U] fp32, padded oh rows
    wt = np.zeros((OHP, OW, K, K, C, U), dtype=np.float32)
    wt[:OH] = w.transpose(0, 1, 3, 4, 5, 2)

    # bias carries the W_SCALE so psum accumulates W_SCALE*(conv+bias);
    # the host gather divides it back out.
    bias_pad = np.zeros((OHP, OW, U), dtype=np.float32)
    bias_pad[:OH] = b * W_SCALE

    in_maps = []
    for core in range(NCORES):
        oh0 = core * NO
        r0 = 2 * oh0
        # x tile: [128, NMP*NR*B]; free = (mp, r, b).
        # Partition halves are PARITY-SWAPPED: tile mp holds its even column
        # (2mp) in partitions 0:64 when mp is even, in partitions 64:128 when
        # mp is odd (legacy layout; main blocks only need the per-pair
        # (dj, c) order, which the flip in the w pack mirrors).
        xc = xT[r0:r0 + NR]                                  # [NR, W, C, B]
        xc = xc.reshape(NR, NMP, 2, C, B)                    # [r, mp, dj, c, b]
        xc = xc.transpose(1, 2, 3, 0, 4)                     # [mp, dj, c, r, b]
        xc = xc.copy()
        xc[1::2] = xc[1::2, ::-1]                            # swap halves, odd mp
        xtile = np.ascontiguousarray(
            xc.transpose(1, 2, 0, 3, 4).reshape(128, NMP * NR * B))

        # x4a: [128=(di,c), OW*5*B]; free = (ow, rp, b); row 2*rp+di of this
        # core, column 2*ow+4. Serves the K=128 j4m blocks (j=4, i=2*?+di).
        rows = xT[r0:r0 + 10].reshape(NO + 1, 2, W, C, B)    # [rp, di, w, c, b]
        cols4 = rows[:, :, 4:4 + 2 * OW:2]                   # [rp, di, ow, c, b]
        x4a = np.ascontiguousarray(
            cols4.transpose(1, 3, 2, 0, 4).reshape(128, OW * (NO + 1) * B))

        # x4b: [128=(dg,c), 15*NO*B]; free = (gp, q, b); row 2*q+4, column
        # 4*gp+4+2*dg. Serves the K=64 corner-tap (i=4, j=4) matmuls.
        rowsb = xT[r0 + 4:r0 + 4 + 2 * NO:2]                 # [q, w, c, b]
        colsb = rowsb[:, 4::2][:, :2 * (OW // 2)]            # [q, g, c, b] g: col 4+2g
        colsb = colsb.reshape(NO, OW // 2, 2, C, B)          # [q, gp, dg, c, b]
        x4b = np.ascontiguousarray(
            colsb.transpose(2, 3, 1, 0, 4).reshape(128, (OW // 2) * NO * B))

        # w stream (built fp32, quantized to e3m4 at the end)
        ws = np.zeros((128, totcols), dtype=np.float32)
        for mp, blocks in enumerate(phases):
            flip = (mp % 2 == 1)
            for bl in blocks:
                c0 = bl["col0"]
                if bl["kind"] == "main":
                    lo, noh = bl["oh0"], bl["noh"]
                    ohs = np.arange(lo, lo + noh)
                    i_s = bl["r"] - 2 * ohs
                    nroles = len(bl["roles"])
                    for k, role in enumerate(bl["roles"]):
                        ow, j0 = mp - role, 2 * role
                        # [noh, 2(dj), C, U]
                        src = wt[oh0 + ohs, ow, i_s, j0:j0 + 2]
                        if flip:
                            src = src[:, ::-1]
                        # -> [128=(dj,c), noh, U]; cols (oh, role, u)-ordered
                        blk = src.transpose(1, 2, 0, 3).reshape(128, noh, U)
                        for t in range(noh):
                            cc = c0 + (t * nroles + k) * U
                            ws[:, cc:cc + U] = blk[:, t]
                elif bl["kind"] == "j4m":
                    ow = bl["ow"]
                    for k, (oh, ib) in enumerate(bl["cons"]):
                        for di in range(2):
                            ws[di * C:(di + 1) * C,
                               c0 + k * U:c0 + (k + 1) * U] = \
                                wt[oh0 + oh, ow, ib + di, 4]
                else:  # j4c
                    gp, q = bl["gp"], bl["q"]
                    ws[0:C, c0:c0 + U] = wt[oh0 + q, 2 * gp, 4, 4]
                    ws[C:2 * C, c0:c0 + U] = wt[oh0 + q, 2 * gp + 1, 4, 4]

        # bias: [1, OW*NO*U] fp32, (ow, oh_l, u) order; broadcast on device
        bias_1 = np.ascontiguousarray(
            bias_pad[oh0:oh0 + NO].transpose(1, 0, 2).reshape(1, OW * NO * U))

        ws8 = (ws * W_SCALE).astype(ml_dtypes.float8_e3m4)
        in_maps.append({"xt": xtile, "x4a": x4a, "x4b": x4b,
                        "wstream": ws8, "bias_1": bias_1})
    return in_maps


def emulate_core(inp):
    """Numpy emulation of the device program for one core (validation)."""
    phases, totcols, _ = build_schedule()
    xt = inp["xt"].astype(np.float32)
    x4a = inp["x4a"].astype(np.float32)
    x4b = inp["x4b"].astype(np.float32)
    ws = inp["wstream"].astype(np.float32)
    bias = np.broadcast_to(inp["bias_1"], (64, OW * NO * U))
    pts = {}
    stag = np.zeros((64, OW, NO, U), dtype=np.float32)
    for mp, blocks in enumerate(phases):
        if mp <= OW:
            pts[mp] = np.zeros((64, NO, 2, U), dtype=np.float32)
        for bl in blocks:
            rhs = ws[:, bl["col0"]:bl["col0"] + bl["ncols"]]
            if bl["kind"] == "main":
                lo, noh = bl["oh0"], bl["noh"]
                xoff = (mp * NR + bl["r"]) * B
                lhsT = xt[:, xoff:xoff + B]
                nroles = len(bl["roles"])
                res = (lhsT.T @ rhs).reshape(64, noh, nroles, U)
                for k, role in enumerate(bl["roles"]):
                    pts[mp][:, lo:lo + noh, role] += res[:, :, k]
            elif bl["kind"] == "j4m":
                ow, rp = bl["ow"], bl["rp"]
                xoff = (ow * (NO + 1) + rp) * B
                lhsT = x4a[:, xoff:xoff + B]
                res = lhsT.T @ rhs                       # [64, ncons*64]
                oh_lo = bl["cons"][0][0]
                nc_ = len(bl["cons"])
                pts[ow][:, oh_lo:oh_lo + nc_, 0] += res.reshape(64, nc_, U)
            else:  # j4c
                gp, q = bl["gp"], bl["q"]
                xoff = (gp * NO + q) * B
                for dg in range(2):
                    lhsT = x4b[dg * C:(dg + 1) * C, xoff:xoff + B]
                    res = lhsT.T @ rhs[dg * C:(dg + 1) * C]
                    pts[2 * gp + dg][:, q, 0] += res
        ow = mp - 2
        if 0 <= ow <= OW - 1:
            bv = bias[:, ow * NO * U:(ow + 1) * NO * U].reshape(64, NO, U)
            stag[:, ow] = pts.pop(ow)[:, :, 0] + pts[ow + 1][:, :, 1] + bv
    return stag / W_SCALE  # [64, ow, oh_l, u]


# ---------------- device kernel ----------------

def build_nc(loop_n=1):
    """Build the device program. loop_n > 1 wraps the whole phase sweep in a
    hardware For_i loop (identical work each iteration) — used only to
    measure per-iteration HW time above the RPC noise floor."""
    import concourse.bass as bass  # noqa: F401
    import concourse.mybir as mybir
    import concourse.tile as tile
    from concourse import bacc

    phases, totcols, wmax = build_schedule()
    dt = mybir.dt

    ablate = os.environ.get("KABLATE", "")  # dev-only: "nomm","nodve","nodma"
    nc = bacc.Bacc("TRN2", target_bir_lowering=False, debug=False,
                   num_devices=NCORES)
    xt_d = nc.dram_tensor("xt", [128, NMP * NR * B], dt.float16,
                          kind="ExternalInput").ap()
    x4a_d = nc.dram_tensor("x4a", [128, OW * (NO + 1) * B], dt.float16,
                           kind="ExternalInput").ap()
    x4b_d = nc.dram_tensor("x4b", [128, (OW // 2) * NO * B], dt.float16,
                           kind="ExternalInput").ap()
    ws_d = nc.dram_tensor("wstream", [128, totcols], dt.float8e3,
                          kind="ExternalInput").ap()
    bias_d = nc.dram_tensor("bias_1", [1, OW * NO * U], dt.float32,
                            kind="ExternalInput").ap()
    out_d = nc.dram_tensor("out", [B, OW, NO, U], dt.bfloat16,
                           kind="ExternalOutput").ap()

    with tile.TileContext(nc) as tc:
        with tc.tile_pool(name="xpool", bufs=1) as xpool, \
             tc.tile_pool(name="bpool", bufs=1) as bpool, \
             tc.tile_pool(name="stpool", bufs=1) as stpool, \
             tc.tile_pool(name="wpool", bufs=int(os.environ.get("WBUFS","8"))) as wpool, \
             tc.tile_pool(name="tmppool", bufs=4) as tmppool, \
             tc.tile_pool(name="pspool", bufs=int(os.environ.get("PSBUFS","5")), space="PSUM") as pspool:

            # Two HWDGE rings: w phase DMAs alternate between them; the x
            # preload is chunked on the ACT ring so early w phases can start
            # while later x chunks stream in.
            dma_w = nc.sync
            dma_x = nc.scalar

            xsb = xpool.tile([128, NMP * NR * B], dt.float16, tag="xt")
            XCH = int(os.environ.get("XCH", "4"))  # x chunks
            xch = NMP // XCH * NR * B
            for g in range(XCH):
                dma_x.dma_start(xsb[:, g * xch:(g + 1) * xch],
                                xt_d[:, g * xch:(g + 1) * xch])
            x4asb = xpool.tile([128, OW * (NO + 1) * B], dt.float16,
                               tag="x4a")
            dma_x.dma_start(x4asb[:, :], x4a_d[:, :])
            x4bsb = xpool.tile([128, (OW // 2) * NO * B], dt.float16,
                               tag="x4b")
            dma_x.dma_start(x4bsb[:, :], x4b_d[:, :])
            b1 = bpool.tile([1, OW * NO * U], dt.float32, tag="b1")
            dma_x.dma_start(b1[:, :], bias_d[:, :])
            bsb = bpool.tile([64, OW * NO * U], dt.float32, tag="brep")
            nc.gpsimd.partition_broadcast(bsb[:, :], b1[:, :], channels=64)
            zsb = bpool.tile([64, NO * 2 * U], dt.float32, tag="zeros")
            nc.vector.memset(zsb[:, :], 0.0)
            stag = stpool.tile([64, OW * NO * U], dt.bfloat16)

            import contextlib
            loop_cm = (tc.For_i(0, loop_n, 1) if loop_n > 1
                       else contextlib.nullcontext())
            with loop_cm:
                _emit_sweep(nc, tc, phases, wmax, dt, ablate, dma_w, dma_x,
                            xsb, x4asb, x4bsb, bsb, zsb, stag, ws_d, out_d,
                            wpool, tmppool, pspool)

    nc.compile()
    return nc


def _emit_sweep(nc, tc, phases, wmax, dt, ablate, dma_w, dma_x,
                xsb, x4asb, x4bsb, bsb, zsb, stag, ws_d, out_d,
                wpool, tmppool, pspool):
    pts = {}
    for mp, blocks in enumerate(phases):
        wcols = sum(bl["ncols"] for bl in blocks)
        if wcols:
            pc0 = blocks[0]["col0"]
            wsb = wpool.tile([128, wmax], dt.float8e3, tag="wstream")
            ring = dma_w if mp % 2 == 0 else dma_x
            if ablate != "nodma":
                ring.dma_start(wsb[:, :wcols],
                               ws_d[:, pc0:pc0 + wcols])
            else:
                # keep a (tiny) writer for the verifier; 99% less traffic
                ring.dma_start(wsb[:, :64], ws_d[:, pc0:pc0 + 64])

        # 2-role PSUM tile [64, 512] = one bank per phase (v1-proven):
        # free layout (oh, role, u); role0 = j01+j4 of loc (oh, mp),
        # role1 = j23 of loc (oh, mp-1). Zeroed on the Activation engine
        # to keep the DVE free for drains.
        if mp <= OW:
            pt = pspool.tile([64, NO * 2 * U], dt.float32)
            pts[mp] = pt
            if ablate != "mmonly":
                nc.scalar.copy(pt[:, :], zsb[:, :])

        def _rv(mp_, role):
            return pts[mp_][:, :].rearrange(
                "p (o r u) -> p o r u", o=NO, r=2, u=U)[:, :, role]

        for bl in blocks:
            loc0 = bl["col0"] - pc0
            rhs = wsb[:, loc0:loc0 + bl["ncols"]]
            if bl["kind"] == "main":
                lo, noh = bl["oh0"], bl["noh"]
                xoff = (mp * NR + bl["r"]) * B
                lhsT = xsb[:, xoff:xoff + B]
                if len(bl["roles"]) == 2:
                    # contiguous (oh, role, u) range
                    outap = pts[mp][:, lo * 2 * U:(lo + noh) * 2 * U]
                else:
                    outap = _rv(mp, bl["roles"][0])[:, lo:lo + noh, :]
                if ablate != "nomm":
                    nc.tensor.matmul(outap, lhsT, rhs, start=False,
                                     stop=False, skip_group_check=True)
            elif bl["kind"] == "j4m":
                ow, rp = bl["ow"], bl["rp"]
                xoff = (ow * (NO + 1) + rp) * B
                lhsT = x4asb[:, xoff:xoff + B]
                oh_lo = bl["cons"][0][0]
                outap = _rv(ow, 0)[:, oh_lo:oh_lo + len(bl["cons"]), :]
                if ablate != "nomm":
                    nc.tensor.matmul(outap, lhsT, rhs, start=False,
                                     stop=False, skip_group_check=True)
            else:  # j4c: corner tap, two K=64 matmuls (ow = 2gp, 2gp+1)
                gp, q = bl["gp"], bl["q"]
                xoff = (gp * NO + q) * B
                for dg in range(2):
                    lhsT = x4bsb[dg * C:(dg + 1) * C, xoff:xoff + B]
                    rhs4 = wsb[dg * C:(dg + 1) * C,
                               loc0:loc0 + U]
                    outap = _rv(2 * gp + dg, 0)[:, q:q + 1, :]
                    if ablate != "nomm":
                        nc.tensor.matmul(outap, lhsT, rhs4, start=False,
                                         stop=False,
                                         skip_group_check=True)

        ow = mp - 2
        if 0 <= ow <= OW - 1:
            a1 = _rv(ow, 0)
            a2 = _rv(ow + 1, 1)
            bv = bsb[:, ow * NO * U:(ow + 1) * NO * U].rearrange(
                "p (o u) -> p o u", u=U)
            stv = stag[:, ow * NO * U:(ow + 1) * NO * U].rearrange(
                "p (o u) -> p o u", u=U)
            if ablate not in ("nodve", "mmonly") or (
                    ablate == "mmonly" and ow == OW - 1):
                tmp = tmppool.tile([64, NO * U], dt.float32)
                tmpv = tmp[:, :].rearrange("p (o u) -> p o u", u=U)
                nc.vector.tensor_add(tmpv, a1, bv)
                nc.vector.tensor_add(stv, tmpv, a2)
            del pts[ow]
            if ablate == "mmonly":
                if ow == OW - 1:  # keep the output written for the verifier
                    sl = slice(ow * NO * U, (ow + 1) * NO * U)
                    dma_w.dma_start(
                        out_d.rearrange("b w o u -> b (w o u)")[:, sl],
                        stag[:, sl])
                continue
            # stream the output out as rows complete: 8-ow chunks early,
            # then 2-ow chunks so the tail DMAs overlap the final drains
            if ow < 24 and ow % 8 == 7:
                g = ow // 8
                sl = slice(g * 8 * NO * U, (g + 1) * 8 * NO * U)
                dma_w.dma_start(
                    out_d.rearrange("b w o u -> b (w o u)")[:, sl],
                    stag[:, sl])
            elif ow >= 24 and ow % 2 == 1:
                sl = slice((ow - 1) * NO * U, (ow + 1) * NO * U)
                dma_w.dma_start(
                    out_d.rearrange("b w o u -> b (w o u)")[:, sl],
                    stag[:, sl])


def _exec(nc, in_maps, repeats=1, chain=1):
    """Execute the prebuilt Bass module on the 8 cores via PJRT/axon.

    Mirrors bass2jax.run_bass_via_pjrt's multi-core branch, but keeps the
    jitted executable + device-staged inputs so the kernel can be re-run for
    timing. `chain` repeats the kernel execution inside one program (for
    amortized on-device timing). Returns (per_core_results, wall_times_s).
    """
    import time

    import jax
    import numpy as _np
    from jax.sharding import Mesh, NamedSharding, PartitionSpec

    try:
        from jax.experimental.shard_map import shard_map
    except ImportError:
        from jax.shard_map import shard_map

    import concourse.mybir as mybir
    from concourse import bass2jax

    bass2jax.install_neuronx_cc_hook()

    partition_name = (nc.partition_id_tensor.name
                      if nc.partition_id_tensor else None)
    in_names, out_names, out_avals, zero_outs = [], [], [], []
    for alloc in nc.m.functions[0].allocations:
        if not isinstance(alloc, mybir.MemoryLocationSet):
            continue
        name = alloc.memorylocations[0].name
        if alloc.kind == "ExternalInput":
            if name != partition_name:
                in_names.append(name)
        elif alloc.kind == "ExternalOutput":
            out_names.append(name)
            shape = tuple(alloc.tensor_shape)
            dtype = mybir.dt.np(alloc.dtype)
            out_avals.append(jax.core.ShapedArray(shape, dtype))
            zero_outs.append(_np.zeros(shape, dtype))
    n_params = len(in_names)
    all_names = in_names + out_names
    if partition_name is not None:
        all_names = all_names + [partition_name]

    def _bind(operands):
        return bass2jax._bass_exec_p.bind(
            *operands,
            out_avals=tuple(out_avals),
            in_names=tuple(all_names),
            out_names=tuple(out_names),
            lowering_input_output_aliases=(),
            sim_require_finite=True,
            sim_require_nnan=True,
            nc=nc,
        )

    def _body(*args):
        operands = list(args)
        if partition_name is not None:
            operands.append(bass2jax.partition_id_tensor())
        return tuple(_bind(operands))

    n_cores = len(in_maps)
    devices = jax.devices()[:n_cores]
    mesh = Mesh(_np.asarray(devices), ("core",))
    spec = PartitionSpec("core")
    sharded = jax.jit(
        shard_map(_body, mesh=mesh, in_specs=(spec,) * (n_params + len(out_names)),
                  out_specs=(spec,) * len(out_names), check_rep=False),
        keep_unused=True,
    )
    sharding = NamedSharding(mesh, spec)
    staged = [
        jax.device_put(
            _np.concatenate([_np.asarray(m[name]) for m in in_maps], axis=0),
            sharding)
        for name in in_names
    ] + [
        jax.device_put(
            _np.zeros((n_cores * z.shape[0], *z.shape[1:]), z.dtype), sharding)
        for z in zero_outs
    ]

    times = []
    out_arrs = None
    for _ in range(max(1, repeats)):
        t0 = time.perf_counter()
        out_arrs = jax.block_until_ready(sharded(*staged))
        times.append(time.perf_counter() - t0)

    results = [
        {
            name: _np.asarray(out_arrs[i]).reshape(n_cores, *out_avals[i].shape)[c]
            for i, name in enumerate(out_names)
        }
        for c in range(n_cores)
    ]
    return results, times


def _run(inputs, repeats=1):
    """Run on hardware. Returns (full_output, wall_times_s)."""
    in_maps = pack_inputs(inputs["x"], inputs["w"], inputs["b"])
    nc = build_nc()
    results, times = _exec(nc, in_maps, repeats=repeats)
    return _gather(results), times


def _gather(results):
    out = np.empty((B, OHP, OW, U), dtype=np.float32)
    for c in range(NCORES):
        # per-core out [B, OW, NO, U] bf16 (scaled by W_SCALE)
        out[:, c * NO:(c + 1) * NO] = (
            results[c]["out"].astype(np.float32).transpose(0, 2, 1, 3))
    return out[:, :OH] * (1.0 / W_SCALE)


def kernel(x, w, b):
    from concourse.bass_utils import run_bass_kernel_spmd

    in_maps = pack_inputs(x, w, b)
    nc = build_nc()
    res = run_bass_kernel_spmd(nc, in_maps, list(range(NCORES)))
    return _gather(res.results)

